# revision 3
# baseline (speedup 1.0000x reference)
"""Trainium2 Bass kernel for nn_Attention_9560597201123 — v2.

Full multi-head attention (B=4, N=2048, E=1024, H=16, D=64), f32 reference.

Sharding: 8 cores = (batch b in 0..4) x (head-half hh in 0..2). Each core:
  - receives x[b].T and the qkv/proj weight slices for its 8 heads
  - computes q/k/v projections for those heads over the full sequence,
    attention for 8 heads, and a PARTIAL output projection (contraction
    over its 512 proj rows), returned as [2048, 1024] f32
Host sums the two partials per batch and adds proj bias.

Device layout notes:
  - heads processed as 4 pairs; pair pr occupies partitions (h_even 0:64 |
    h_odd 64:128) of qt/kt/aT
  - scores computed transposed (S^T: keys on partitions, queries free); the
    two heads of a pair run as row-tiled (tile_position) K=64 matmuls that
    stream CONCURRENTLY through the PE array
  - softmax normalizer: v is extended with a ones column (lhsT M=65), so
    the PV matmul's partition 64 accumulates the row sums for free
  - part of the exp work runs on VectorE as a 16-bit Schraudolph fast-exp
    (affine + f32->i16 convert, bits reinterpreted as bf16), the rest on
    ScalarE's exact exp; ScalarE would otherwise be the bottleneck
  - q/k biases via activation bias on eviction; v bias via an extra K=1
    ones-row matmul accumulated into the projection
"""

import numpy as np
import ml_dtypes

P = 128
SEQ = 2048
E = 1024
HH = 8  # heads per core
NP = 4  # head pairs per core
D = 64
KC = 16  # key chunks of 128
EC = 8  # e_in chunks of 128
SCALE = D ** -0.5  # 0.125

# Schraudolph bf16 fast-exp constants (exp(SCALE*s) via bf16 bit trick)
FEXP_C = 128.0 / float(np.log(2.0)) * SCALE
FEXP_B = 16256.0 - 5.5

# kc positions whose exp runs on VectorE (fast-exp) instead of ScalarE
DVE_KCS = (1, 3, 5, 8, 10, 12)
DEBUG_DUMP = False

_NC = None


def build_nc():
    global _NC
    if _NC is not None:
        return _NC

    import concourse.bass as bass  # noqa: F401
    import concourse.mybir as mybir
    import concourse.tile as tile
    from concourse import bacc

    BF = mybir.dt.bfloat16
    F32 = mybir.dt.float32
    I16 = mybir.dt.int16
    EXP = mybir.ActivationFunctionType.Exp
    LOG = mybir.ActivationFunctionType.Ln
    COPY = mybir.ActivationFunctionType.Identity
    ADD = mybir.AluOpType.add
    MULT = mybir.AluOpType.mult

    nc = bacc.Bacc("TRN2", target_bir_lowering=False, debug=False, num_devices=8)

    xt_d = nc.dram_tensor("xt", [E, SEQ], BF, kind="ExternalInput").ap()
    wq_d = nc.dram_tensor("wq", [E, 512], BF, kind="ExternalInput").ap()
    wk_d = nc.dram_tensor("wk", [E, 512], BF, kind="ExternalInput").ap()
    wv_d = nc.dram_tensor("wv", [E, 512], BF, kind="ExternalInput").ap()
    bqk_d = nc.dram_tensor("bqk", [P, 8], F32, kind="ExternalInput").ap()
    bv_d = nc.dram_tensor("bv", [512], BF, kind="ExternalInput").ap()
    wpt_d = nc.dram_tensor("wpt", [P, NP, E], BF, kind="ExternalInput").ap()
    out_d = nc.dram_tensor("out", [SEQ, E], F32, kind="ExternalOutput").ap()
    if DEBUG_DUMP:
        qt0_d = nc.dram_tensor("qt0_dbg", [P, SEQ], BF, kind="ExternalOutput").ap()
        kt0_d = nc.dram_tensor("kt0_dbg", [P, SEQ], BF, kind="ExternalOutput").ap()
        at0_d = nc.dram_tensor("at0_dbg", [P, SEQ], BF, kind="ExternalOutput").ap()
        vx_d = nc.dram_tensor("vx_dbg", [P, KC * HH * 65], BF, kind="ExternalOutput").ap()
        sc0_d = nc.dram_tensor("sc0_dbg", [P, 1024], F32, kind="ExternalOutput").ap()
        asb0_d = nc.dram_tensor("asb0_dbg", [65, 512], F32, kind="ExternalOutput").ap()
        rr0_d = nc.dram_tensor("rr0_dbg", [64, 512], F32, kind="ExternalOutput").ap()
        rs0_d = nc.dram_tensor("rs0_dbg", [1, 512], F32, kind="ExternalOutput").ap()

    wq_r = wq_d.rearrange("(o p) c -> p o c", p=P)
    wk_r = wk_d.rearrange("(o p) c -> p o c", p=P)
    wv_r = wv_d.rearrange("(o p) c -> p o c", p=P)

    with tile.TileContext(nc) as tc:
        with (
            tc.tile_pool(name="persist", bufs=1) as persist,
            tc.tile_pool(name="wstream", bufs=2) as wstream,
            tc.tile_pool(name="ptpool", bufs=6) as ptpool,
            tc.tile_pool(name="pt16pool", bufs=4) as pt16pool,
            tc.tile_pool(name="asbp", bufs=4) as asbp,
            tc.tile_pool(name="small", bufs=2) as small,
            tc.tile_pool(name="ysbp", bufs=2) as ysbp,
            tc.tile_pool(name="sc_ps", bufs=3, space="PSUM") as sc_ps,
            tc.tile_pool(name="acc_ps", bufs=2, space="PSUM") as acc_ps,
        ):
            # ---- persistent tiles + input DMA ----
            xt_r = xt_d.rearrange("(o p) s -> p o s", p=P)
            xt3 = persist.tile([P, EC, SEQ], BF, tag="xt")
            for qq in range(4):
                nc.sync.dma_start(
                    xt3[:, :, qq * 512 : (qq + 1) * 512],
                    xt_r[:, :, qq * 512 : (qq + 1) * 512],
                )
            xt = [xt3[:, ec, :] for ec in range(EC)]

            # v (+ones) per key chunk: [keys, head, 65]
            vx = persist.tile([P, KC, HH, 65], BF, tag="vx", name="vx")
            nc.vector.memset(vx[:, :, :, 64], 1.0)

            kt = [persist.tile([P, SEQ], BF, tag=f"kt{p}", name=f"kt{p}") for p in range(NP)]
            qt = [persist.tile([P, SEQ], BF, tag=f"qt{p}", name=f"qt{p}") for p in range(NP)]
            aT = [persist.tile([P, SEQ], BF, tag=f"aT{p}", name=f"aT{p}") for p in range(NP)]

            bqk = persist.tile([P, 8], F32, tag="bqk")
            nc.scalar.dma_start(bqk[:], bqk_d[:])
            bv_row = persist.tile([1, 512], BF, tag="bv_row")
            nc.scalar.dma_start(bv_row[:], bv_d[None])
            bv_bc = persist.tile([P, 512], BF, tag="bv_bc")
            nc.gpsimd.partition_broadcast(bv_bc[:], bv_row[:])

            wpt = persist.tile([P, NP, E], BF, tag="wpt")
            nc.sync.dma_start(wpt[:], wpt_d[:])

            # ---- emitters ----
            def emit_qk_dma(pr):
                wqs = wstream.tile([P, EC, P], BF, tag="wq")
                nc.gpsimd.dma_start(wqs[:], wq_r[:, :, pr * P : (pr + 1) * P])
                wks = wstream.tile([P, EC, P], BF, tag="wk")
                nc.gpsimd.dma_start(wks[:], wk_r[:, :, pr * P : (pr + 1) * P])
                return wqs, wks

            def emit_proj_tile(w, dst, qh, bias_col):
                # one [128, 1024] psum tile = queries qh*1024..+1024 of one
                # 128-dim projection chunk; two 512-col accumulation groups
                ps = sc_ps.tile([P, 1024], F32, tag="sc", name="ps_proj")
                for half in range(2):
                    qsl = slice(qh * 1024 + half * 512, qh * 1024 + (half + 1) * 512)
                    for ec in range(EC):
                        nc.tensor.matmul(
                            ps[:, half * 512 : (half + 1) * 512],
                            lhsT=w[:, ec, :],
                            rhs=xt[ec][:, qsl],
                            start=(ec == 0),
                            stop=(ec == EC - 1),
                        )
                nc.scalar.activation(
                    out=dst[:, qh * 1024 : (qh + 1) * 1024],
                    in_=ps[:],
                    func=COPY,
                    bias=bqk[:, bias_col : bias_col + 1],
                )

            def emit_v_tile(wvs, vt):
                # one [128, 1024] psum tile = two key chunks (2vt, 2vt+1) x
                # all 512 v columns of this core's heads
                ps = sc_ps.tile([P, 1024], F32, tag="sc", name="ps_v")
                for half in range(2):
                    kc = 2 * vt + half
                    osl = slice(half * 512, (half + 1) * 512)
                    for ec in range(EC):
                        nc.tensor.matmul(
                            ps[:, osl],
                            lhsT=xt[ec][:, kc * P : (kc + 1) * P],
                            rhs=wvs[:, ec, :],
                            start=(ec == 0),
                            stop=(ec == EC - 1),
                        )
                    nc.vector.tensor_tensor(
                        out=vx[:, 2 * vt + half, :, 0:64],
                        in0=ps[:, osl].rearrange("p (h c) -> p h c", c=64),
                        in1=bv_bc[:].rearrange("p (h c) -> p h c", c=64),
                        op=ADD,
                    )

            pending_norm = []

            def emit_att(pr, qh, callbacks):
                # PV runs one kc behind scores so the exp latency hides
                # under the next kc's score stream
                cbs = list(callbacks)
                for qb in (2 * qh, 2 * qh + 1):
                    qsl = slice(qb * 512, (qb + 1) * 512)
                    accA = acc_ps.tile([65, 512], F32, tag="acc", name="accA")
                    accB = acc_ps.tile([65, 512], F32, tag="acc", name="accB")
                    prev_ptv = None

                    def emit_pv(ptv, kc, accA=accA, accB=accB, pr=pr):
                        nc.tensor.matmul(
                            accA[:],
                            lhsT=vx[:, kc, 2 * pr, :],
                            rhs=ptv[:, 0:512],
                            start=(kc == 0),
                            stop=(kc == KC - 1),
                        )
                        nc.tensor.matmul(
                            accB[:],
                            lhsT=vx[:, kc, 2 * pr + 1, :],
                            rhs=ptv[:, 512:1024],
                            start=(kc == 0),
                            stop=(kc == KC - 1),
                        )

                    def emit_sc_exp(kc):
                        ksl = slice(kc * P, (kc + 1) * P)
                        sc = sc_ps.tile([P, 1024], F32, tag="sc", name="sc")
                        nc.tensor.matmul(
                            sc[:, 0:512],
                            lhsT=kt[pr][0:64, ksl],
                            rhs=qt[pr][0:64, qsl],
                            tile_position=(0, 0),
                        )
                        nc.tensor.matmul(
                            sc[:, 512:1024],
                            lhsT=kt[pr][64:P, ksl],
                            rhs=qt[pr][64:P, qsl],
                            tile_position=(64, 0),
                        )
                        if DEBUG_DUMP and pr == 0 and qb == 0 and kc == 0:
                            sdbg = ysbp.tile([P, 1024], F32, tag="scdbg", bufs=1)
                            nc.vector.tensor_copy(out=sdbg[:], in_=sc[:])
                            nc.sync.dma_start(sc0_d[:], sdbg[:])
                        if kc in DVE_KCS:
                            pt16 = pt16pool.tile([P, 1024], I16, tag="pt16", name="pt16")
                            nc.vector.tensor_scalar(
                                out=pt16[:],
                                in0=sc[:],
                                scalar1=FEXP_C,
                                scalar2=FEXP_B,
                                op0=MULT,
                                op1=ADD,
                            )
                            return pt16[:].bitcast(BF)
                        pt = ptpool.tile([P, 1024], BF, tag="pt", name="pt")
                        nc.scalar.activation(out=pt[:], in_=sc[:], func=EXP, scale=SCALE)
                        return pt[:]

                    pend = []
                    for i in range(KC // 4):
                        for j in range(4):
                            pend.append((4 * i + j, emit_sc_exp(4 * i + j)))
                        if i > 0:
                            for kcx, ptv in pend[:4]:
                                emit_pv(ptv, kcx)
                            pend = pend[4:]
                        if i in (1, 2, 3) and cbs:
                            cbs.pop(0)()
                    for kcx, ptv in pend:
                        emit_pv(ptv, kcx)
                    for hh_odd, acc in ((0, accA), (1, accB)):
                        asb = asbp.tile([65, 512], F32, tag="asb")
                        nc.vector.tensor_copy(out=asb[:], in_=acc[:])
                        if DEBUG_DUMP and pr == 0 and qb == 0 and hh_odd == 0:
                            nc.sync.dma_start(asb0_d[:], asb[:])
                        pending_norm.append((asb, hh_odd, pr, qsl))
                # run any leftover callbacks (shouldn't normally happen)
                for cb in cbs:
                    cb()

            def flush_norm_pieces():
                # batched normalize, split into 3 schedulable pieces: gather
                # the sum rows into one packed tile (DMA moves partitions),
                # ONE partition-parallel DVE reciprocal for the whole batch,
                # then broadcast+multiply per entry
                batch = list(pending_norm)
                pending_norm.clear()
                if not batch:
                    return []
                state = {}

                def cb_gather():
                    spk = small.tile([4, 512], F32, tag="spk")
                    for i, (asb, _, _, _) in enumerate(batch):
                        nc.sync.dma_start(spk[i : i + 1, :], asb[64:65, :])
                    state["spk"] = spk

                def cb_recip():
                    rp = small.tile([4, 512], F32, tag="rp")
                    nc.vector.reciprocal(rp[0 : len(batch), :], state["spk"][0 : len(batch), :])
                    state["rp"] = rp

                def cb_apply():
                    for i, (asb, hh_odd, pr, qsl) in enumerate(batch):
                        rpi = small.tile([1, 512], F32, tag="rpi", bufs=4)
                        nc.sync.dma_start(rpi[:], state["rp"][i : i + 1, :])
                        R = small.tile([64, 512], F32, tag="R")
                        nc.gpsimd.partition_broadcast(R[:], rpi[:])
                        if hh_odd == 0:
                            nc.vector.tensor_tensor(
                                out=aT[pr][0:64, qsl],
                                in0=asb[0:64, :],
                                in1=R[:],
                                op=MULT,
                            )
                        else:
                            tmpb = small.tile([64, 512], BF, tag="tmpb")
                            nc.vector.tensor_tensor(
                                out=tmpb[:], in0=asb[0:64, :], in1=R[:], op=MULT
                            )
                            nc.sync.dma_start(aT[pr][64:P, qsl], tmpb[:])

                return [cb_gather, cb_recip, cb_apply]

            def emit_out_proj(qc):
                yps = sc_ps.tile([P, 1024], F32, tag="sc", name="yps")
                qcs = slice(qc * P, (qc + 1) * P)
                for nh in range(2):
                    nsl = slice(nh * 512, (nh + 1) * 512)
                    for pp in range(NP):
                        nc.tensor.matmul(
                            yps[:, nsl],
                            lhsT=aT[pp][:, qcs],
                            rhs=wpt[:, pp, nsl],
                            start=(pp == 0),
                            stop=(pp == NP - 1),
                        )
                ysb = ysbp.tile([P, 1024], F32, tag="ysb")
                nc.scalar.activation(out=ysb[:], in_=yps[:], func=COPY)
                nc.sync.dma_start(out_d[qcs, :], ysb[:])

            # ---- main schedule ----
            wqs0, wks0 = emit_qk_dma(0)
            emit_proj_tile(wqs0, qt[0], 0, 0)
            emit_proj_tile(wqs0, qt[0], 1, 0)
            emit_proj_tile(wks0, kt[0], 0, 4)
            emit_proj_tile(wks0, kt[0], 1, 4)
            with tc.tile_pool(name="wvpool", bufs=1) as wvpool:
                wvs = wvpool.tile([P, EC, 512], BF, tag="wv")
                nc.gpsimd.dma_start(wvs[:], wv_r[:])
                for vt in range(5):
                    emit_v_tile(wvs, vt)

                for pr in range(NP):
                    if pr < NP - 1:
                        nxt = pr + 1
                        wqs, wks = emit_qk_dma(nxt)
                        if pr == 0:
                            cbs_qh0 = [
                                lambda: emit_v_tile(wvs, 5),
                                lambda: emit_v_tile(wvs, 6),
                                lambda: emit_v_tile(wvs, 7),
                            ] + [
                                lambda w=wqs, n=nxt: emit_proj_tile(w, qt[n], 0, n),
                                lambda w=wqs, n=nxt: emit_proj_tile(w, qt[n], 1, n),
                            ]
                        else:
                            cbs_qh0 = flush_norm_pieces() + [
                                lambda w=wqs, n=nxt: emit_proj_tile(w, qt[n], 0, n),
                                lambda w=wqs, n=nxt: emit_proj_tile(w, qt[n], 1, n),
                            ]
                        emit_att(pr, 0, cbs_qh0)
                        cbs_qh1 = flush_norm_pieces() + [
                            lambda w=wks, n=nxt: emit_proj_tile(w, kt[n], 0, 4 + n),
                            lambda w=wks, n=nxt: emit_proj_tile(w, kt[n], 1, 4 + n),
                        ]
                        emit_att(pr, 1, cbs_qh1)
                    else:
                        cbs_qh0 = flush_norm_pieces()
                        emit_att(pr, 0, cbs_qh0)
                        cbs_qh1 = flush_norm_pieces() + [
                            lambda qc=qc: emit_out_proj(qc) for qc in range(3)
                        ]
                        emit_att(pr, 1, cbs_qh1)

            # tail: out-proj for qhalf0 leftovers first (they only need qh0
            # norms), interleaved with the final norm flush, then qhalf1
            tail_flush = flush_norm_pieces()
            emit_out_proj(3)
            if tail_flush:
                tail_flush[0]()
            emit_out_proj(4)
            if tail_flush:
                tail_flush[1]()
            emit_out_proj(5)
            emit_out_proj(6)
            if tail_flush:
                tail_flush[2]()
            emit_out_proj(7)
            for qc in range(8, 16):
                emit_out_proj(qc)

            if DEBUG_DUMP:
                nc.sync.dma_start(qt0_d[:], qt[0][:])
                nc.sync.dma_start(kt0_d[:], kt[0][:])
                nc.sync.dma_start(at0_d[:], aT[0][:])
                nc.sync.dma_start(vx_d[:], vx[:].rearrange("p k h c -> p (k h c)"))

    nc.finalize()
    _NC = nc
    return nc


def make_in_maps(x, qkv_w, qkv_b, proj_w, proj_b):
    bf16 = ml_dtypes.bfloat16
    x = np.asarray(x, dtype=np.float32)
    qkv_w = np.asarray(qkv_w, dtype=np.float32)
    qkv_b = np.asarray(qkv_b, dtype=np.float32)
    proj_w = np.asarray(proj_w, dtype=np.float32)
    in_maps = []
    for c in range(8):
        b, hh = divmod(c, 2)
        cs = slice(hh * 512, (hh + 1) * 512)
        bqk = np.empty((P, 8), dtype=np.float32)
        for pr in range(4):
            bqk[:, pr] = qkv_b[hh * 512 + pr * P : hh * 512 + (pr + 1) * P]
            bqk[:, 4 + pr] = qkv_b[E + hh * 512 + pr * P : E + hh * 512 + (pr + 1) * P]
        wp_slice = proj_w[hh * 512 : (hh + 1) * 512, :]
        wpt = np.ascontiguousarray(
            wp_slice.reshape(4, 2, 64, E).transpose(1, 2, 0, 3).reshape(P, 4, E)
        )
        in_maps.append(
            {
                "xt": np.ascontiguousarray(x[b].T).astype(bf16),
                "wq": np.ascontiguousarray(qkv_w[:, cs]).astype(bf16),
                "wk": np.ascontiguousarray(qkv_w[:, E + hh * 512 : E + (hh + 1) * 512]).astype(bf16),
                "wv": np.ascontiguousarray(
                    qkv_w[:, 2 * E + hh * 512 : 2 * E + (hh + 1) * 512]
                ).astype(bf16),
                "bqk": bqk,
                "bv": qkv_b[2 * E + hh * 512 : 2 * E + (hh + 1) * 512].astype(bf16),
                "wpt": wpt.astype(bf16),
            }
        )
    return in_maps


def assemble_out(results, proj_b):
    out = np.empty((4, SEQ, E), dtype=np.float32)
    pb = np.asarray(proj_b, dtype=np.float32)
    for b in range(4):
        out[b] = results[2 * b]["out"] + results[2 * b + 1]["out"] + pb
    return out


def run(inputs, trace=False):
    """Run on 8 NeuronCores; returns (output, BassKernelResults)."""
    from concourse.bass_utils import run_bass_kernel_spmd

    nc = build_nc()
    in_maps = make_in_maps(**inputs)
    res = run_bass_kernel_spmd(nc, in_maps, core_ids=list(range(8)), trace=trace)
    return assemble_out(res.results, inputs["proj_b"]), res


def kernel(x, qkv_w, qkv_b, proj_w, proj_b):
    inputs = dict(x=x, qkv_w=qkv_w, qkv_b=qkv_b, proj_w=proj_w, proj_b=proj_b)
    out, _ = run(inputs, trace=False)
    for _ in range(2):
        # guard against rare transient device faults: recompute if corrupted
        if np.isfinite(out).all():
            break
        out, _ = run(inputs, trace=False)
    return out


if __name__ == "__main__":
    rng = np.random.default_rng(0)
    x = rng.standard_normal((4, SEQ, E), dtype=np.float32)
    s = E ** -0.5
    inputs = dict(
        x=x,
        qkv_w=rng.standard_normal((E, 3 * E), dtype=np.float32) * s,
        qkv_b=rng.standard_normal((3 * E,), dtype=np.float32) * 0.02,
        proj_w=rng.standard_normal((E, E), dtype=np.float32) * s,
        proj_b=rng.standard_normal((E,), dtype=np.float32) * 0.02,
    )
    out = kernel(**inputs)
    print("out", out.shape, out.dtype, float(np.abs(out).mean()))
